# revision 6
# baseline (speedup 1.0000x reference)
"""MinVQVAE multi-query forward on 8 trn2 NeuronCores, data-parallel over batch.

Per core (batch shard 512, feature-major activations):
  enc (fp32 PE):  h1=gelu(W1 h0), h2=gelu(W2 h1), zeT=W3 h2 + b3  -> DRAM
  score (fp32):   per query: scores[b,c] = zeT_q.T @ epT, argmax via DVE max/max_index
  gather (f32r):  one-hot mask from idx, zqT = ep.T @ mask; S2 = sum((ze-zq)^2)
  dec (f32r PE):  g1=gelu(D1 zq), g2=gelu(D2 g1), xpT=sigmoid(D3 g2); S1 = sum((x-xp)^2)
Host: shard/transpose inputs, one-hot z_discrete from idx, combine loss partials.
"""
from contextlib import ExitStack

import numpy as np

import concourse.mybir as mybir
from concourse import bacc
from concourse.tile import TileContext
from concourse.masks import make_identity

F32 = mybir.dt.float32
F32R = mybir.dt.float32r
U32 = mybir.dt.uint32
AF = mybir.ActivationFunctionType
OP = mybir.AluOpType

P = 128

# full-size problem dims
B, INPUT_DIM, N_CAT, D_LAT, N_Q, N_HID = 4096, 3072, 1024, 256, 32, 4096
N_CORES = 8
PB = B // N_CORES  # batch per core (= matmul moving dim)

W_CHUNK = 4  # k-subtiles per weight DMA chunk


def _mlp_layer(nc, tc, ctx, name, K, M, w_dram, bias_sb, rhs_big, out_big,
               act, dtype, out_stage_dram=None, xin_dram=None, s1_acc=None):
    """outT[M, PB] = act(wT.T @ rhsT + bias).

    rhs_big: SBUF tile [P, K//P, PB] (dtype matching w)
    out_big: SBUF tile [P, M//P, PB] or None (then stage to out_stage_dram)
    If xin_dram/s1_acc given (dec3): accumulate sum((x - out)^2) into s1_acc cols.
    """
    KT, MT = K // P, M // P
    n_groups = MT // 4
    wp = ctx.enter_context(tc.tile_pool(name=f"{name}_w", bufs=3))
    pp = ctx.enter_context(tc.tile_pool(name=f"{name}_ps", bufs=8, space="PSUM"))
    sp = None
    if out_stage_dram is not None:
        sp = ctx.enter_context(tc.tile_pool(name=f"{name}_st", bufs=3))
    for g in range(n_groups):
        psums = [pp.tile([P, PB], F32, tag="ps", name=f"ps{i}") for i in range(4)]
        for kc0 in range(0, KT, W_CHUNK):
            kn = min(W_CHUNK, KT - kc0)
            wt = wp.tile([P, W_CHUNK, 512], dtype, tag="w")
            nc.sync.dma_start(
                wt[:, :kn, :],
                w_dram[kc0 * P:(kc0 + kn) * P, g * 512:(g + 1) * 512]
                .rearrange("(s p) m -> p s m", p=P))
            for kk in range(kn):
                k = kc0 + kk
                for ms in range(4):
                    nc.tensor.matmul(psums[ms][:],
                                     wt[:, kk, ms * P:(ms + 1) * P],
                                     rhs_big[:, k, :],
                                     start=(k == 0), stop=(k == KT - 1))
        for ms in range(4):
            m = g * 4 + ms
            if out_big is not None:
                nc.scalar.activation(out_big[:, m, :], psums[ms][:], act,
                                     bias=bias_sb[:, m:m + 1])
            else:
                st_dt = F32 if xin_dram is not None else dtype
                st = sp.tile([P, PB], st_dt, tag="st")
                nc.scalar.activation(st[:], psums[ms][:], act,
                                     bias=bias_sb[:, m:m + 1])
                nc.sync.dma_start(out_stage_dram[m * P:(m + 1) * P, :], st[:])
                if xin_dram is not None:
                    xt = sp.tile([P, PB], F32, tag="xt")
                    nc.sync.dma_start(xt[:], xin_dram[m * P:(m + 1) * P, :])
                    d = sp.tile([P, PB], F32, tag="d")
                    nc.vector.tensor_sub(d[:], xt[:], st[:])
                    sq = sp.tile([P, PB], F32, tag="sq")
                    nc.vector.tensor_mul(sq[:], d[:], d[:])
                    nc.vector.reduce_sum(s1_acc[:, m:m + 1], sq[:],
                                         axis=mybir.AxisListType.X)


def build(nc=None):
    if nc is None:
        nc = bacc.Bacc(trn_type="TRN2", name="vqvae")
    NHT, ZT = N_HID // P, (N_Q * D_LAT) // P
    KT1 = INPUT_DIM // P        # 24
    DLT = D_LAT // P            # 2
    CT = N_CAT // P             # 8
    XT_T = INPUT_DIM // P       # 24
    BS = PB // P                # 4 batch subtiles

    # ---- I/O ----
    xT = nc.dram_tensor("xT", [INPUT_DIM, PB], F32, kind="ExternalInput")
    w1T = nc.dram_tensor("w1T", [INPUT_DIM, N_HID], F32, kind="ExternalInput")
    w2T = nc.dram_tensor("w2T", [N_HID, N_HID], F32, kind="ExternalInput")
    w3T = nc.dram_tensor("w3T", [N_HID, N_Q * D_LAT], F32, kind="ExternalInput")
    d1T = nc.dram_tensor("d1T", [N_Q * D_LAT, N_HID], F32R, kind="ExternalInput")
    d2T = nc.dram_tensor("d2T", [N_HID, N_HID], F32R, kind="ExternalInput")
    d3T = nc.dram_tensor("d3T", [N_HID, INPUT_DIM], F32R, kind="ExternalInput")
    b1 = nc.dram_tensor("b1", [P, NHT], F32, kind="ExternalInput")
    b2 = nc.dram_tensor("b2", [P, NHT], F32, kind="ExternalInput")
    b3 = nc.dram_tensor("b3", [P, ZT], F32, kind="ExternalInput")
    db1 = nc.dram_tensor("db1", [P, NHT], F32, kind="ExternalInput")
    db2 = nc.dram_tensor("db2", [P, NHT], F32, kind="ExternalInput")
    db3 = nc.dram_tensor("db3", [P, XT_T], F32, kind="ExternalInput")
    epT = nc.dram_tensor("epT", [D_LAT, N_CAT], F32, kind="ExternalInput")
    ep = nc.dram_tensor("ep", [N_CAT, D_LAT], F32R, kind="ExternalInput")
    ciota = nc.dram_tensor("ciota", [P, CT], F32, kind="ExternalInput")

    xpT_out = nc.dram_tensor("xpT_out", [INPUT_DIM, PB], F32, kind="ExternalOutput")
    idx_out = nc.dram_tensor("idx_out", [PB, N_Q], F32, kind="ExternalOutput")
    s1_out = nc.dram_tensor("s1_out", [P, 1], F32, kind="ExternalOutput")
    s2_out = nc.dram_tensor("s2_out", [P, 1], F32, kind="ExternalOutput")

    zeT_dram = nc.dram_tensor("zeT_scratch", [N_Q * D_LAT, PB], F32, kind="Internal")
    g1_dram = nc.dram_tensor("g1_scratch", [N_HID, PB], F32R, kind="Internal")

    with TileContext(nc) as tc:
        with (
            tc.tile_pool(name="persist", bufs=1) as pers,
            tc.tile_pool(name="acc", bufs=1) as accp,
        ):
            ident = pers.tile([P, P], F32, tag="ident")
            make_identity(nc, ident)
            ciota_sb = pers.tile([P, CT], F32, tag="ciota")
            nc.sync.dma_start(ciota_sb[:], ciota[:, :])
            bias = {}
            for nm, dram, cols in [("b1", b1, NHT), ("b2", b2, NHT),
                                   ("b3", b3, ZT), ("db1", db1, NHT),
                                   ("db2", db2, NHT), ("db3", db3, XT_T)]:
                t = pers.tile([P, cols], F32, tag=nm)
                nc.sync.dma_start(t[:], dram[:, :])
                bias[nm] = t
            idx_all = pers.tile([P, BS, N_Q], F32, tag="idx_all")
            idxT_sb = pers.tile([N_Q, BS, P], F32, tag="idxT")
            s1_acc = accp.tile([P, XT_T], F32, tag="s1a")
            s2_acc = accp.tile([P, N_Q * DLT], F32, tag="s2a")

            # ================= encoder =================
            with tc.tile_pool(name="hpool1", bufs=1) as hp1:
                h1 = hp1.tile([P, NHT, PB], F32, tag="h1")
                with tc.tile_pool(name="xpool", bufs=1) as xp_pool:
                    x_big = xp_pool.tile([P, KT1, PB], F32, tag="x")
                    nc.sync.dma_start(
                        x_big[:], xT.rearrange("(s p) n -> p s n", p=P))
                    with ExitStack() as el:
                        _mlp_layer(nc, tc, el, "enc1", INPUT_DIM, N_HID, w1T,
                                   bias["b1"], x_big, h1, AF.Gelu, F32)
                with tc.tile_pool(name="hpool2", bufs=1) as hp2:
                    h2 = hp2.tile([P, NHT, PB], F32, tag="h2")
                    with ExitStack() as el:
                        _mlp_layer(nc, tc, el, "enc2", N_HID, N_HID, w2T,
                                   bias["b2"], h1, h2, AF.Gelu, F32)
                    with ExitStack() as el:
                        _mlp_layer(nc, tc, el, "enc3", N_HID, N_Q * D_LAT, w3T,
                                   bias["b3"], h2, None, AF.Identity, F32,
                                   out_stage_dram=zeT_dram)

            # ================= scoring + argmax =================
            with ExitStack() as sc:
                cp = sc.enter_context(tc.tile_pool(name="sc_c", bufs=1))
                epT_sb = cp.tile([P, DLT, N_CAT], F32, tag="epT")
                nc.sync.dma_start(
                    epT_sb[:], epT.rearrange("(s p) c -> p s c", p=P))
                zp = sc.enter_context(tc.tile_pool(name="sc_ze", bufs=3))
                pp = sc.enter_context(tc.tile_pool(name="sc_ps", bufs=3,
                                                   space="PSUM"))
                sp = sc.enter_context(tc.tile_pool(name="sc_s", bufs=3))
                mp = sc.enter_context(tc.tile_pool(name="sc_m", bufs=4))
                for q in range(N_Q):
                    ze_q = zp.tile([P, DLT, PB], F32, tag="ze")
                    nc.sync.dma_start(
                        ze_q[:], zeT_dram[q * D_LAT:(q + 1) * D_LAT, :]
                        .rearrange("(s p) n -> p s n", p=P))
                    for bs in range(BS):
                        ps = pp.tile([P, N_CAT], F32, tag="ps")
                        for ch in range(2):
                            for ds in range(DLT):
                                nc.tensor.matmul(
                                    ps[:, ch * 512:(ch + 1) * 512],
                                    ze_q[:, ds, bs * P:(bs + 1) * P],
                                    epT_sb[:, ds, ch * 512:(ch + 1) * 512],
                                    start=(ds == 0), stop=(ds == DLT - 1))
                        s_sb = sp.tile([P, N_CAT], F32, tag="s")
                        nc.vector.tensor_copy(s_sb[:], ps[:])
                        mx = mp.tile([P, 8], F32, tag="mx")
                        mi = mp.tile([P, 8], U32, tag="mi")
                        nc.vector.max(out=mx[:], in_=s_sb[:])
                        nc.vector.max_index(out=mi[:], in_max=mx[:],
                                            in_values=s_sb[:])
                        nc.vector.tensor_copy(idx_all[:, bs, q:q + 1], mi[:, 0:1])
                nc.sync.dma_start(
                    idx_out.rearrange("(s p) q -> p s q", p=P), idx_all[:])
                # transpose idx [128b, NQ] -> [NQ, 128b] per batch subtile
                tpp = sc.enter_context(tc.tile_pool(name="sc_tp", bufs=2,
                                                    space="PSUM"))
                for bs in range(BS):
                    tp = tpp.tile([N_Q, P], F32, tag="tp")
                    nc.tensor.transpose(tp[:], idx_all[:, bs, :], ident[:])
                    nc.vector.tensor_copy(idxT_sb[:, bs, :], tp[:])

            # ================= gather + S2 =================
            with tc.tile_pool(name="zq", bufs=1) as zq_pool:
                zq_big = zq_pool.tile([P, ZT, PB], F32R, tag="zq")
                with ExitStack() as ga:
                    cp = ga.enter_context(tc.tile_pool(name="ga_c", bufs=1))
                    ep_sb = cp.tile([P, CT, D_LAT], F32R, tag="ep")
                    nc.sync.dma_start(
                        ep_sb[:], ep.rearrange("(s p) d -> p s d", p=P))
                    ones_t = cp.tile([1, P], F32, tag="ones")
                    nc.vector.memset(ones_t[:], 1.0)
                    stp = ga.enter_context(tc.tile_pool(name="ga_st", bufs=2))
                    bps = ga.enter_context(tc.tile_pool(name="ga_bps", bufs=2,
                                                        space="PSUM"))
                    qps = ga.enter_context(tc.tile_pool(name="ga_qps", bufs=4,
                                                        space="PSUM"))
                    mkp = ga.enter_context(tc.tile_pool(name="ga_mk", bufs=2))
                    for q in range(N_Q):
                        stage = stp.tile([1, PB], F32, tag="stg")
                        nc.sync.dma_start(stage[:], idxT_sb[q:q + 1, :, :])
                        bps_t = bps.tile([P, PB], F32, tag="bc")
                        nc.tensor.matmul(bps_t[:], ones_t[:], stage[:],
                                         start=True, stop=True)
                        mask = mkp.tile([P, CT, PB], F32R, tag="mk")
                        for cs in range(CT):
                            nc.vector.tensor_scalar(
                                mask[:, cs, :], bps_t[:], ciota_sb[:, cs:cs + 1],
                                None, op0=OP.is_equal)
                        for ds in range(DLT):
                            qp = qps.tile([P, PB], F32, tag="qp")
                            for cs in range(CT):
                                nc.tensor.matmul(
                                    qp[:], ep_sb[:, cs, ds * P:(ds + 1) * P],
                                    mask[:, cs, :],
                                    start=(cs == 0), stop=(cs == CT - 1))
                            nc.vector.tensor_copy(zq_big[:, q * DLT + ds, :], qp[:])

                # ===== S2 = sum((ze - zq)^2), zq resident =====
                with ExitStack() as s2s:
                    zep = s2s.enter_context(tc.tile_pool(name="s2_ze", bufs=3))
                    ZT_ = (N_Q * D_LAT) // P
                    for zs in range(ZT_):
                        ze_t = zep.tile([P, PB], F32, tag="ze")
                        nc.sync.dma_start(ze_t[:],
                                          zeT_dram[zs * P:(zs + 1) * P, :])
                        d = zep.tile([P, PB], F32, tag="d")
                        nc.vector.tensor_sub(d[:], ze_t[:],
                                             zq_big[:, zs, :].bitcast(F32))
                        sq = zep.tile([P, PB], F32, tag="sq")
                        nc.vector.tensor_mul(sq[:], d[:], d[:])
                        nc.vector.reduce_sum(s2_acc[:, zs:zs + 1], sq[:],
                                             axis=mybir.AxisListType.X)

                # ================= dec1 (zq resident) =================
                with ExitStack() as el:
                    _mlp_layer(nc, tc, el, "dec1", N_Q * D_LAT, N_HID, d1T,
                               bias["db1"], zq_big, None, AF.Gelu, F32R,
                               out_stage_dram=g1_dram)

            # ================= dec2 / dec3 =================
            with (
                tc.tile_pool(name="g1", bufs=1) as g1p,
                tc.tile_pool(name="g2", bufs=1) as g2p,
            ):
                g1 = g1p.tile([P, NHT, PB], F32R, tag="g1")
                nc.sync.dma_start(
                    g1[:], g1_dram.rearrange("(s p) n -> p s n", p=P))
                g2 = g2p.tile([P, NHT, PB], F32R, tag="g2")
                with ExitStack() as el:
                    _mlp_layer(nc, tc, el, "dec2", N_HID, N_HID, d2T,
                               bias["db2"], g1, g2, AF.Gelu, F32R)
                with ExitStack() as el:
                    _mlp_layer(nc, tc, el, "dec3", N_HID, INPUT_DIM, d3T,
                               bias["db3"], g2, None, AF.Sigmoid, F32R,
                               out_stage_dram=xpT_out, xin_dram=xT,
                               s1_acc=s1_acc)

            # ================= finalize losses =================
            fin = accp.tile([P, 2], F32, tag="fin")
            nc.vector.reduce_sum(fin[:, 0:1], s1_acc[:], axis=mybir.AxisListType.X)
            nc.vector.reduce_sum(fin[:, 1:2], s2_acc[:], axis=mybir.AxisListType.X)
            nc.sync.dma_start(s1_out[:, :], fin[:, 0:1])
            nc.sync.dma_start(s2_out[:, :], fin[:, 1:2])

    return nc


# ======================= host side =======================
_CACHE = {}


def _prep(inputs):
    def t(a):
        return np.ascontiguousarray(np.asarray(a, np.float32).T)

    def bcol(b, cols):
        return np.ascontiguousarray(np.asarray(b, np.float32).reshape(cols, P).T)

    shared = {
        "w1T": t(inputs["ew1"]), "w2T": t(inputs["ew2"]), "w3T": t(inputs["ew3"]),
        "d1T": t(inputs["dw1"]), "d2T": t(inputs["dw2"]), "d3T": t(inputs["dw3"]),
        "b1": bcol(inputs["eb1"], N_HID // P), "b2": bcol(inputs["eb2"], N_HID // P),
        "b3": bcol(inputs["eb3"], (N_Q * D_LAT) // P),
        "db1": bcol(inputs["db1"], N_HID // P), "db2": bcol(inputs["db2"], N_HID // P),
        "db3": bcol(inputs["db3"], INPUT_DIM // P),
        "epT": t(inputs["embed_pool"]),
        "ep": np.ascontiguousarray(np.asarray(inputs["embed_pool"], np.float32)),
        "ciota": np.ascontiguousarray(
            np.arange(N_CAT, dtype=np.float32).reshape(N_CAT // P, P).T),
    }
    x = np.asarray(inputs["x"], np.float32)
    in_maps = []
    for c in range(N_CORES):
        m = dict(shared)
        m["xT"] = np.ascontiguousarray(x[c * PB:(c + 1) * PB].T)
        in_maps.append(m)
    return in_maps


def run(inputs, trace=False):
    from concourse import bass_utils
    if "nc" not in _CACHE:
        nc = build()
        nc.finalize()
        _CACHE["nc"] = nc
    nc = _CACHE["nc"]
    in_maps = _prep(inputs)
    res = bass_utils.run_bass_kernel_spmd(
        nc, in_maps, core_ids=list(range(N_CORES)), trace=trace)
    return res


def postprocess(results, x):
    x_pred = np.empty((B, INPUT_DIM), np.float32)
    idx = np.empty((B, N_Q), np.int64)
    s1 = 0.0
    s2 = 0.0
    for c, r in enumerate(results):
        x_pred[c * PB:(c + 1) * PB] = r["xpT_out"].T
        idx[c * PB:(c + 1) * PB] = r["idx_out"].astype(np.int64)
        s1 += float(r["s1_out"].astype(np.float64).sum())
        s2 += float(r["s2_out"].astype(np.float64).sum())
    z_discrete = np.zeros((B, N_Q, N_CAT), np.int32)
    bi = np.arange(B)[:, None]
    qi = np.arange(N_Q)[None, :]
    z_discrete[bi, qi, idx] = 1
    loss = (s1 / (B * INPUT_DIM) + 1.25 * s2 / (B * N_Q * D_LAT)) / B
    return x_pred, z_discrete, np.float32(loss)


def kernel(**inputs):
    res = run(inputs, trace=False)
    return postprocess(res.results, inputs["x"])
